# revision 12
# baseline (speedup 1.0000x reference)
"""Trainium2 Bass kernel for the LoRA dynamics MLP.

Math: out = L2(relu(L1(relu(L0(concat(state, action))))))
with Li(x) = x @ (Wi + s*Ui@Di).T + bi  (LoRA folded into the base GEMM,
exact algebra: x@W.T + s*(x@Di.T)@Ui.T == x@(W + s*Ui@Di).T).

Distribution: pure data parallel over 8 NeuronCores (batch 65536 -> 8192
rows/core); the folded weights are replicated.

Device layout: activations are feature-major ([features, batch]); every
layer is psum[mj] = sum_k WT[k, mj].T @ xT[k, :]. All matmul operands are
fp16 (1 PE cycle/row, same as fp32r at N>=512, but half the HBM traffic);
PSUM accumulates fp32; ScalarE fuses bias+ReLU (PSUM->SBUF, fp16 out);
DVE adds the last-layer bias and the output is DMA'd out as fp16 (upcast
on host).

The batch loop is software-pipelined three deep -- iteration i runs
L0(i), L1(i-1), L2(i-2) -- so every cross-engine dependency (ScalarE
activation feeding the next layer's matmuls) has a full iteration
(~6.5us) of slack and the tensor engine never stalls.  That matters
beyond overlap: the cost model's PE p-state ramp only reaches full clock
(2.4 GHz) after 3us of *gapless* execution, and any stall resets it to
1.2 GHz.  A short warmup matmul chain on scratch data starts the ramp
while the first input tiles stream in.
"""

import numpy as np

import concourse.mybir as mybir
import concourse.tile as tile
from concourse import bacc, bass_utils

P = 128
B = 65536
S = 768
A = 128
F0 = S + A            # 896
H = 256
NCORES = 8
BC = B // NCORES      # 8192 rows per core
BT = 512              # batch tile (matmul moving dim; one PSUM bank fp32)
NBT = BC // BT        # 16 batch tiles per core
KO0, KO1, KO2 = F0 // P, H // P, H // P     # 7, 2, 2 contraction tiles
MO0, MO1, MO2 = H // P, H // P, S // P      # 2, 2, 6 output tiles
LORA_SCALE = 16.0 / 8.0

F32 = mybir.dt.float32
F16 = mybir.dt.float16
RELU = mybir.ActivationFunctionType.Relu

_NC_CACHE = []
LAST_RESULT = None  # BassKernelResults of the most recent run (for test.py)


def _build(xp_bufs=3, h_bufs=3, yp_bufs=3, in_split=2, out_split=2,
           warmup_mm=8, pa_bufs=2, pb_bufs=2, pc_bufs=4,
           last_split=None, w0_split=1, x0_on_act=False, last_alt=False,
           y_on_pool=False):
    nc = bacc.Bacc("TRN2", target_bir_lowering=False, debug=False,
                   num_devices=NCORES)
    xT = nc.dram_tensor("xT", [F0, BC], F16, kind="ExternalInput").ap()
    w0t = nc.dram_tensor("w0t", [F0, H], F16, kind="ExternalInput").ap()
    w1t = nc.dram_tensor("w1t", [H, H], F16, kind="ExternalInput").ap()
    w2t = nc.dram_tensor("w2t", [H, S], F16, kind="ExternalInput").ap()
    b0 = nc.dram_tensor("b0", [H], F32, kind="ExternalInput").ap()
    b1 = nc.dram_tensor("b1", [H], F32, kind="ExternalInput").ap()
    b2 = nc.dram_tensor("b2", [S], F32, kind="ExternalInput").ap()
    yT = nc.dram_tensor("yT", [S, BC], F16, kind="ExternalOutput").ap()

    with tile.TileContext(nc) as tc:
        with (
            tc.tile_pool(name="wp", bufs=1) as wp,
            tc.tile_pool(name="xp", bufs=xp_bufs) as xp,
            tc.tile_pool(name="hp", bufs=h_bufs) as hp,
            tc.tile_pool(name="pa", bufs=pa_bufs, space="PSUM") as pa,
            tc.tile_pool(name="pb", bufs=pb_bufs, space="PSUM") as pb,
            tc.tile_pool(name="pc", bufs=pc_bufs, space="PSUM") as pc,
            tc.tile_pool(name="yp", bufs=yp_bufs) as yp,
        ):
            # -- weights / biases (replicated, loaded once) --
            w0_sb = wp.tile([P, KO0, H], F16)
            w0_r = w0t.rearrange("(ko p) m -> p ko m", p=P)
            for g in range(w0_split):
                ks = slice(g * KO0 // w0_split, (g + 1) * KO0 // w0_split)
                nc.sync.dma_start(w0_sb[:, ks, :], w0_r[:, ks, :])
            b0_sb = wp.tile([P, MO0], F32)
            nc.sync.dma_start(b0_sb[:], b0.rearrange("(mo p) -> p mo", p=P))
            w1_sb = wp.tile([P, KO1, H], F16)
            w2_sb = wp.tile([P, KO2, S], F16)
            b1_sb = wp.tile([P, MO1], F32)
            b2_sb = wp.tile([P, MO2], F32)

            def load_rest():
                nc.sync.dma_start(w1_sb[:], w1t.rearrange("(ko p) m -> p ko m", p=P))
                nc.sync.dma_start(w2_sb[:], w2t.rearrange("(ko p) m -> p ko m", p=P))
                nc.sync.dma_start(b1_sb[:], b1.rearrange("(mo p) -> p mo", p=P))
                nc.sync.dma_start(b2_sb[:], b2.rearrange("(mo p) -> p mo", p=P))

            xT_t = xT.rearrange("(ko p) b -> p ko b", p=P)
            yT_t = yT.rearrange("(mo p) b -> p mo b", p=P)

            # -- PE p-state warmup: a matmul chain on zeroed scratch keeps
            # the tensor engine ramping while the first tiles stream in --
            if warmup_mm:
                scratch = wp.tile([P, BT], F16)
                nc.vector.memset(scratch[:], 0.0)
                wps = pa.tile([P, BT], F32, tag="ps")
                for n in range(warmup_mm):
                    nc.tensor.matmul(wps[:], scratch[:, 0:P], scratch[:],
                                     start=(n == 0), stop=(n == warmup_mm - 1))

            h1_tiles = {}
            h2_tiles = {}

            for i in range(NBT + 2):
                # ---- stage A: input DMA + layer 0 for batch tile i ----
                if i < NBT:
                    bsl = slice(i * BT, (i + 1) * BT)
                    x_sb = xp.tile([P, KO0, BT], F16, tag="x")
                    x_eng = nc.scalar if (x0_on_act and i == 0) else nc.sync
                    for g in range(in_split):
                        ks = slice(g * KO0 // in_split,
                                   (g + 1) * KO0 // in_split)
                        x_eng.dma_start(x_sb[:, ks, :], xT_t[:, ks, bsl])

                    h1 = hp.tile([P, KO1, BT], F16, tag="h1")
                    h1_tiles[i] = h1
                    for mj in range(MO0):
                        ps = pa.tile([P, BT], F32, tag="ps")
                        for k in range(KO0):
                            nc.tensor.matmul(ps[:],
                                             w0_sb[:, k, mj * P:(mj + 1) * P],
                                             x_sb[:, k, :],
                                             start=(k == 0),
                                             stop=(k == KO0 - 1))
                        nc.scalar.activation(h1[:, mj, :], ps[:], RELU,
                                             bias=b0_sb[:, mj:mj + 1],
                                             scale=1.0)
                    if i == 0:
                        load_rest()

                # ---- stage B: layer 1 for batch tile i-1 ----
                j = i - 1
                if 0 <= j < NBT:
                    h1 = h1_tiles.pop(j)
                    h2 = hp.tile([P, KO2, BT], F16, tag="h2")
                    h2_tiles[j] = h2
                    for mj in range(MO1):
                        ps = pb.tile([P, BT], F32, tag="ps")
                        for k in range(KO1):
                            nc.tensor.matmul(ps[:],
                                             w1_sb[:, k, mj * P:(mj + 1) * P],
                                             h1[:, k, :],
                                             start=(k == 0),
                                             stop=(k == KO1 - 1))
                        nc.scalar.activation(h2[:, mj, :], ps[:], RELU,
                                             bias=b1_sb[:, mj:mj + 1],
                                             scale=1.0)

                # ---- stage C: layer 2 + output DMA for batch tile i-2 ----
                kk = i - 2
                if 0 <= kk < NBT:
                    bslk = slice(kk * BT, (kk + 1) * BT)
                    h2 = h2_tiles.pop(kk)
                    y_sb = yp.tile([P, MO2, BT], F16, tag="y")
                    osplit = out_split
                    if last_split is not None and kk == NBT - 1:
                        osplit = last_split
                    if isinstance(osplit, list):
                        bounds = osplit
                    else:
                        bounds = [MO2 * (g + 1) // osplit
                                  for g in range(osplit)]
                    for mj in range(MO2):
                        ps = pc.tile([P, BT], F32, tag="ps")
                        for k in range(KO2):
                            nc.tensor.matmul(ps[:],
                                             w2_sb[:, k, mj * P:(mj + 1) * P],
                                             h2[:, k, :],
                                             start=(k == 0),
                                             stop=(k == KO2 - 1))
                        nc.vector.tensor_tensor(
                            y_sb[:, mj, :], ps[:],
                            b2_sb[:, mj:mj + 1].to_broadcast((P, BT)),
                            mybir.AluOpType.add)
                        if (mj + 1) in bounds:
                            gi = bounds.index(mj + 1)
                            lo = 0 if gi == 0 else bounds[gi - 1]
                            msl = slice(lo, mj + 1)
                            y_eng = nc.gpsimd if y_on_pool else nc.scalar
                            if last_alt and kk == NBT - 1:
                                # final tile: alternate sequencers, ending
                                # on the otherwise-idle SP queue
                                nlast = len(bounds)
                                y_eng = (nc.sync if (nlast - 1 - gi) % 2 == 0
                                         else (nc.gpsimd if y_on_pool
                                               else nc.scalar))
                            y_eng.dma_start(yT_t[:, msl, bslk],
                                            y_sb[:, msl, :])
    nc.compile()
    return nc


def kernel(state, action, W0, b0, W1, b1, W2, b2,
           D0, U0, D1, U1, D2, U2):
    global LAST_RESULT
    state = np.asarray(state, dtype=np.float32)
    action = np.asarray(action, dtype=np.float32)

    def fold(W, U, D):
        # exact LoRA merge in float64; rounded to fp16 for the PE
        We = W.astype(np.float64) + LORA_SCALE * (
            U.astype(np.float64) @ D.astype(np.float64))
        return np.ascontiguousarray(We.T.astype(np.float16))  # [in, out]

    w0t = fold(np.asarray(W0), np.asarray(U0), np.asarray(D0))
    w1t = fold(np.asarray(W1), np.asarray(U1), np.asarray(D1))
    w2t = fold(np.asarray(W2), np.asarray(U2), np.asarray(D2))
    b0 = np.ascontiguousarray(np.asarray(b0, dtype=np.float32))
    b1 = np.ascontiguousarray(np.asarray(b1, dtype=np.float32))
    b2 = np.ascontiguousarray(np.asarray(b2, dtype=np.float32))

    # feature-major fp16 input, sharded over cores along batch
    xT = np.empty((F0, B), dtype=np.float16)
    xT[:S] = state.T
    xT[S:] = action.T

    if not _NC_CACHE:
        _NC_CACHE.append(_build())
    nc = _NC_CACHE[0]

    in_maps = [
        {
            "xT": np.ascontiguousarray(xT[:, c * BC:(c + 1) * BC]),
            "w0t": w0t, "w1t": w1t, "w2t": w2t,
            "b0": b0, "b1": b1, "b2": b2,
        }
        for c in range(NCORES)
    ]
    res = bass_utils.run_bass_kernel_spmd(nc, in_maps,
                                          core_ids=list(range(NCORES)))
    LAST_RESULT = res

    out = np.empty((B, S), dtype=np.float32)
    for c in range(NCORES):
        out[c * BC:(c + 1) * BC, :] = res.results[c]["yT"].T
    return out
